# revision 18
# baseline (speedup 1.0000x reference)
"""GCN layer on 8 TRN2 NeuronCores (Bass/Tile).

out = segment_sum(edge_vals[:,None] * (X @ W)[edge_col], edge_row, N)

Strategy (1D destination-node sharding, v2):
  - Host: cast/transpose X -> XT bf16 (replicated to all 8 cores). Partition
    edges by destination shard (6250 rows/core) and group by destination
    window (128 rows). One unified gather stream per core (no lo/hi split):
    the XW table packs TWO nodes per 256-byte row (node u in cols 0:64,
    node u+25088 in cols 64:128), so gather indices fit int16 and the
    parity select is done on the consumer side by scaling the two column
    halves with host-precomputed valsE/valsO (val where parity matches,
    else 0) and summing.
  - Device phase 1: XW = X @ W computed redundantly per core (TensorE bf16,
    fp32 PSUM), tiles computed in (lo i, hi i+196) pairs so each 256 B table
    row is written contiguously; stored bf16 to DRAM in partition-major
    order (row r at virtual row (r%128)*196 + r//128) via large DMAs.
  - Device phase 2: dma_gather (SWDGE) fetches table rows per edge; the
    descriptor generation is split round-robin across 4 SWDGE queues so all
    8 GPSIMD cores generate descriptors in parallel (queue q runs on Q7
    cores 2q, 2q+1). VectorE builds S[e, r] = (row_local[e] == r) via an
    iota compare (bf16 meta for 2x DVE rate) and the parity-selected,
    val-scaled rhs; TensorE accumulates S^T @ rhs into the window's PSUM
    [128, 64]. Windows are written out dense - no scatter races anywhere.
  - Host: concatenate the 8 output shards.
"""

from contextlib import ExitStack

import ml_dtypes
import numpy as np

import concourse.bacc as bacc
import concourse.bass as bass
import concourse.mybir as mybir
import concourse.tile as tile
from concourse._compat import get_trn_type
from concourse.bass_utils import run_bass_kernel_spmd

N_NODES = 50000
N_EDGES = 800000
F_IN = 256
F_OUT = 64
N_CORES = 8
SHARD = N_NODES // N_CORES  # 6250 destination rows per core
WIN = 64  # destination rows per PSUM accumulation window
BF16 = ml_dtypes.bfloat16

HALF = 25088  # node-pair split: row r holds node r (cols 0:64) and r+HALF
NTL = HALF // 128  # 196 pair-tiles; table rows = HALF, all int16-addressable

# knobs
SLAB = 2048  # phase-1 node columns per XT slab DMA (per lo/hi stream)
GRP = 28  # phase-1 pair tiles per staged XW store DMA (196 = 7*28)
CH = 16  # phase-2 edge tiles (of 128 edges) per dma_gather call
GB = 16  # phase-2 edge tiles per batched one-hot / rhs build (divides CH)
NQ = 4  # SWDGE queues used round-robin for gather desc-gen
SIM_MEMSET = False  # zero staging tiles (only needed to appease CoreSim)

# test.py pokes these for profiling
TRACE = False
LAST_RESULTS = None


def _install_ntff_hook():
    """The agent image's antenv lacks axon_hooks, so bass_utils' trace=True
    path can't find the NTFF hook. Recreate the module and register the
    ctypes-based hook exactly as trn_agent_boot would."""
    import sys
    import types

    try:
        import antenv.axon_hooks  # noqa: F401

        return True
    except ImportError:
        pass
    try:
        import antenv
        from trn_agent_boot.trn_boot import _ntff_profile_via_ctypes

        mod = types.ModuleType("antenv.axon_hooks")
        mod._hook = None

        def set_axon_ntff_profile_hook(h):
            mod._hook = h

        def get_axon_ntff_profile_hook():
            return mod._hook

        mod.set_axon_ntff_profile_hook = set_axon_ntff_profile_hook
        mod.get_axon_ntff_profile_hook = get_axon_ntff_profile_hook
        sys.modules["antenv.axon_hooks"] = mod
        antenv.axon_hooks = mod
        hook = _ntff_profile_via_ctypes("/opt/axon/libaxon_pjrt.so")
        if hook is not None:
            set_axon_ntff_profile_hook(hook)
        return hook is not None
    except Exception as e:  # profiling is best-effort
        print(f"ntff hook install failed: {e}")
        return False


def _wrap16(stream_i16, n_tiles):
    """Wrapped+replicated dma_gather index layout: stream position i lives at
    partition i%16 (replicated to all 8 16-partition groups), slot i//16."""
    n = n_tiles * 128
    w = np.zeros((128, n // 16), dtype=np.int16)
    s = np.zeros(n, dtype=np.int16)
    s[: len(stream_i16)] = stream_i16
    blk = s.reshape(n // 16, 16).T  # [16, n//16]
    for g in range(8):
        w[g * 16 : (g + 1) * 16, :] = blk
    return w


def _prep(X, W, edge_row, edge_col, edge_vals):
    """Host-side sharding/marshalling.

    Returns (in_maps, T): per-window tile counts (maxed across cores so all
    8 cores run the identical SPMD program).
    """
    XT = np.ascontiguousarray(X.T).astype(BF16)  # [F_IN, N_NODES]
    Wb = np.ascontiguousarray(W).astype(BF16)  # [F_IN, F_OUT]
    # iota_big[p, r*GB + j] = r: a REAL (non-broadcast) operand for the
    # [128, WIN, GB]-layout one-hot build, so both tensor_tensor inputs
    # stream with unit inner stride.
    iota = np.tile(np.repeat(np.arange(WIN, dtype=np.float32), GB), (128, 1))

    n_win = (SHARD + WIN - 1) // WIN  # 49
    core = edge_row // SHARD
    percore = []
    cnt = np.zeros((N_CORES, n_win), dtype=np.int64)
    for p in range(N_CORES):
        m = core == p
        r = edge_row[m].astype(np.int64) - p * SHARD
        c = edge_col[m].astype(np.int64)
        v = edge_vals[m].astype(np.float32)
        w = r // WIN
        tr = c % HALF  # table row
        par = (c >= HALF).astype(np.float32)  # 0 -> cols 0:64, 1 -> 64:128
        q = (tr % 128) * NTL + tr // 128  # partition-major virtual row
        # sort by (window, q): monotone gather addresses within each window
        # give the HBM controller sequential-ish access patterns.
        order = np.lexsort((q, w))
        r, q, v, par, w = r[order], q[order], v[order], par[order], w[order]
        percore.append((r, q, v, par, w))
        cnt[p] = np.bincount(w, minlength=n_win)

    T = np.maximum(1, -(-cnt.max(axis=0) // 128))
    J0 = int(T.sum())
    J = -(-J0 // CH) * CH  # pad to whole gather chunks
    starts = np.concatenate([[0], np.cumsum(T)])

    in_maps = []
    for p in range(N_CORES):
        r, q, v, par, w = percore[p]
        qbuf = np.zeros(J * 128, dtype=np.int64)
        rowloc = np.zeros(J * 128, dtype=np.float32)
        valsE = np.zeros(J * 128, dtype=np.float32)
        valsO = np.zeros(J * 128, dtype=np.float32)
        wstart = np.searchsorted(w, np.arange(n_win))
        wend = np.searchsorted(w, np.arange(n_win), side="right")
        for wi in range(n_win):
            a, b = int(wstart[wi]), int(wend[wi])
            n = b - a
            s0 = int(starts[wi]) * 128
            qbuf[s0 : s0 + n] = q[a:b]
            rowloc[s0 : s0 + n] = (r[a:b] % WIN).astype(np.float32)
            valsE[s0 : s0 + n] = v[a:b] * (1.0 - par[a:b])
            valsO[s0 : s0 + n] = v[a:b] * par[a:b]
        meta = np.concatenate(
            [
                rowloc.reshape(J, 128).T,
                valsE.reshape(J, 128).T,
                valsO.reshape(J, 128).T,
                iota,
            ],
            axis=1,
        ).astype(np.float32)
        in_maps.append(
            {
                "xt": XT,
                "w": Wb,
                "cols": _wrap16(qbuf.astype(np.int16), J),
                "meta": np.ascontiguousarray(meta),
            }
        )
    return in_maps, T


def _build_nc(T, n_nodes=N_NODES, f_in=F_IN, f_out=F_OUT, shard=SHARD):
    f32 = mybir.dt.float32
    bf16 = mybir.dt.bfloat16
    i16 = mybir.dt.int16
    n_win = len(T)
    J0 = int(T.sum())
    J = -(-J0 // CH) * CH
    starts = np.concatenate([[0], np.cumsum(T)])
    n_hi = n_nodes - HALF  # 24912 real hi nodes

    nc = bacc.Bacc(
        get_trn_type() or "TRN2",
        target_bir_lowering=False,
        dynamic_dma_scratch_size=32768,
        num_swdge_queues=NQ,
    )
    xt = nc.dram_tensor("xt", [f_in, n_nodes], bf16, kind="ExternalInput")
    w_in = nc.dram_tensor("w", [f_in, f_out], bf16, kind="ExternalInput")
    cols = nc.dram_tensor("cols", [128, J * 8], i16, kind="ExternalInput")
    meta = nc.dram_tensor("meta", [128, 3 * J + GB * WIN], f32, kind="ExternalInput")
    out = nc.dram_tensor("out", [shard, f_out], f32, kind="ExternalOutput")
    # XW table: virtual row b*NTL + a holds table row r = 128*a + b, which
    # packs node r (cols 0:64) and node r+HALF (cols 64:128) -> 256 B rows.
    xw = nc.dram_tensor("xw", [HALF, 128], bf16, kind="Internal")

    n_kc = f_in // 128  # contraction chunks (2)

    with tile.TileContext(nc) as tc, ExitStack() as ctx:
        const = ctx.enter_context(tc.tile_pool(name="const", bufs=1))
        xt_pool = ctx.enter_context(tc.tile_pool(name="xtp", bufs=2))
        psum1 = ctx.enter_context(tc.tile_pool(name="psum1", bufs=4, space="PSUM"))
        xw_sb = ctx.enter_context(tc.tile_pool(name="xw_sb", bufs=2))
        gath = [
            ctx.enter_context(tc.tile_pool(name=f"gath{qi}", bufs=3))
            for qi in range(NQ)
        ]
        s_pool = ctx.enter_context(tc.tile_pool(name="s_pool", bufs=3))
        rhs_pool = ctx.enter_context(tc.tile_pool(name="rhs_pool", bufs=3))
        tmp_pool = ctx.enter_context(tc.tile_pool(name="tmp_pool", bufs=3))
        psum2 = ctx.enter_context(tc.tile_pool(name="psum2", bufs=4, space="PSUM"))
        out_sb = ctx.enter_context(tc.tile_pool(name="out_sb", bufs=4))

        # resident constants
        w_t = []
        for k in range(n_kc):
            wt = const.tile([128, f_out], bf16, tag=f"w{k}")
            nc.sync.dma_start(out=wt[:], in_=w_in[k * 128 : (k + 1) * 128, :])
            w_t.append(wt)
        meta_t = const.tile([128, 3 * J + GB * WIN], f32, tag="meta")
        nc.sync.dma_start(out=meta_t[:], in_=meta[:, :])
        cols_t = const.tile([128, J * 8], i16, tag="cols")
        nc.sync.dma_start(out=cols_t[:], in_=cols[:, :])

        # shared num_idxs register for all gather calls (one MOVE total)
        nreg = nc.gpsimd.to_reg(CH * 128)

        # ---- phase 1: xw table, computed in (lo i, hi i+NTL) pairs ----
        xw_pm = xw[:, :].rearrange("(b a) f -> b (a f)", b=128)  # [128, NTL*128]
        stg = None
        g0 = 0
        ps = None
        xtl = xth = None
        s_lo = s_hi = 0
        for i in range(NTL):
            if i % (SLAB // 128) == 0:
                s_lo = i * 128
                s_hi = HALF + i * 128
                sl = min(SLAB, HALF - s_lo)
                sh = min(SLAB, n_nodes - s_hi)
                xtl, xth = [], []
                for k in range(n_kc):
                    t1 = xt_pool.tile([128, SLAB], bf16, tag=f"xtl{k}")
                    nc.sync.dma_start(
                        out=t1[:, :sl], in_=xt[k * 128 : (k + 1) * 128, s_lo : s_lo + sl]
                    )
                    xtl.append(t1)
                    if sh > 0:
                        t2 = xt_pool.tile([128, SLAB], bf16, tag=f"xth{k}")
                        nc.sync.dma_start(
                            out=t2[:, :sh],
                            in_=xt[k * 128 : (k + 1) * 128, s_hi : s_hi + sh],
                        )
                        xth.append(t2)
            if i % GRP == 0:
                g0 = i
                stg = xw_sb.tile([128, GRP * 128], bf16, tag="stg")
                if SIM_MEMSET:
                    nc.gpsimd.memset(stg[:], 0)
            if i % 2 == 0:
                ps = psum1.tile([128, 256], f32, tag="ps1")
            off = (i % 2) * 128
            lo0 = i * 128 - s_lo
            for k in range(n_kc):
                nc.tensor.matmul(
                    out=ps[:, off : off + f_out],
                    lhsT=xtl[k][:, lo0 : lo0 + 128],
                    rhs=w_t[k][:],
                    start=(k == 0),
                    stop=(k == n_kc - 1),
                )
            m_hi = min(128, n_hi - i * 128)
            if m_hi > 0:
                hi0 = (HALF + i * 128) - s_hi
                for k in range(n_kc):
                    nc.tensor.matmul(
                        out=ps[:m_hi, off + f_out : off + 128],
                        lhsT=xth[k][:, hi0 : hi0 + m_hi],
                        rhs=w_t[k][:],
                        start=(k == 0),
                        stop=(k == n_kc - 1),
                    )
            if i % 2 == 1 or i == NTL - 1:
                loc = ((i - 1 if i % 2 == 1 else i) - g0) * 128
                ncols = 256 if i % 2 == 1 else 128
                dst = stg[:, loc : loc + ncols]
                if (i // 2) % 2 == 0:
                    nc.scalar.activation(
                        out=dst, in_=ps[:, :ncols],
                        func=mybir.ActivationFunctionType.Copy,
                    )
                else:
                    nc.vector.tensor_copy(out=dst, in_=ps[:, :ncols])
            if i == NTL - 1 or (i + 1) % GRP == 0:
                gn = i + 1 - g0
                nc.sync.dma_start(
                    out=xw_pm[:, g0 * 128 : (g0 + gn) * 128], in_=stg[:, : gn * 128]
                )

        # ---- phase 2: multi-queue dma_gather + one-hot matmul segment-sum ----
        chunks = {}
        batches = {}

        def ensure_chunk(tile_idx):
            ci = tile_idx // CH
            if ci in chunks:
                return chunks[ci]
            q = ci % NQ
            g = gath[q].tile([128, CH, 128], bf16, tag=f"g{q}")
            nc.gpsimd.dma_gather(
                out_ap=g[:, :, :],
                in_ap=xw[:, :],
                idxs_ap=cols_t[:, ci * CH * 8 : (ci + 1) * CH * 8],
                num_idxs=CH * 128,
                num_idxs_reg=nreg,
                elem_size=128,
                single_packet=False,
                queue_num=q,
            )
            chunks[ci] = g
            return g

        def ensure_batch(tile_idx):
            bi = tile_idx // GB
            if bi in batches:
                return batches[bi]
            b0 = bi * GB
            g = ensure_chunk(b0)
            gs = b0 - (b0 // CH) * CH
            S_b = s_pool.tile([128, WIN, GB], bf16, tag="S")
            rhs_b = rhs_pool.tile([128, GB, f_out], bf16, tag="rhs")
            tmp_b = tmp_pool.tile([128, GB, f_out], bf16, tag="tmp")
            nc.vector.tensor_tensor(
                out=S_b[:],
                in0=meta_t[:, 3 * J : 3 * J + WIN * GB].rearrange(
                    "p (r b) -> p r b", b=GB
                ),
                in1=meta_t[:, b0 : b0 + GB]
                .rearrange("p (one b) -> p one b", one=1)
                .to_broadcast([128, WIN, GB]),
                op=mybir.AluOpType.is_equal,
            )
            nc.vector.tensor_tensor(
                out=rhs_b[:],
                in0=g[:, gs : gs + GB, 0:f_out],
                in1=meta_t[:, J + b0 : J + b0 + GB].to_broadcast([128, GB, f_out]),
                op=mybir.AluOpType.mult,
            )
            nc.vector.tensor_tensor(
                out=tmp_b[:],
                in0=g[:, gs : gs + GB, f_out:128],
                in1=meta_t[:, 2 * J + b0 : 2 * J + b0 + GB].to_broadcast(
                    [128, GB, f_out]
                ),
                op=mybir.AluOpType.mult,
            )
            batches[bi] = (S_b, rhs_b, tmp_b)
            return batches[bi]

        for w in range(n_win):
            cur_ps = psum2.tile([WIN, f_out], f32, tag="ps2")
            n_t = int(T[w])
            for k in range(n_t):
                t_s = int(starts[w]) + k
                S_b, rhs_b, tmp_b = ensure_batch(t_s)
                sl = t_s % GB
                lhsT = S_b[:, :, sl : sl + 1].rearrange("p r one -> p (r one)")
                nc.tensor.matmul(
                    out=cur_ps[:],
                    lhsT=lhsT,
                    rhs=rhs_b[:, sl : sl + 1, :],
                    start=(k == 0),
                    stop=False,
                )
                nc.tensor.matmul(
                    out=cur_ps[:],
                    lhsT=lhsT,
                    rhs=tmp_b[:, sl : sl + 1, :],
                    start=False,
                    stop=(k == n_t - 1),
                )
            rows = min(WIN, shard - w * WIN)
            ot = out_sb.tile([WIN, f_out], f32, tag="ot")
            nc.vector.tensor_copy(out=ot[:rows, :], in_=cur_ps[:rows, :])
            nc.sync.dma_start(out=out[w * WIN : w * WIN + rows, :], in_=ot[:rows, :])
    nc.compile()
    return nc


def kernel(X, W, edge_row, edge_col, edge_vals):
    global LAST_RESULTS
    X = np.asarray(X, dtype=np.float32)
    W = np.asarray(W, dtype=np.float32)
    edge_row = np.asarray(edge_row, dtype=np.int32)
    edge_col = np.asarray(edge_col, dtype=np.int32)
    edge_vals = np.asarray(edge_vals, dtype=np.float32)

    in_maps, T = _prep(X, W, edge_row, edge_col, edge_vals)
    nc = _build_nc(T)
    trace = TRACE and _install_ntff_hook()
    res = run_bass_kernel_spmd(
        nc, in_maps, core_ids=list(range(N_CORES)), trace=trace
    )
    LAST_RESULTS = res
    out = np.concatenate([res.results[p]["out"] for p in range(N_CORES)], axis=0)
    return out.astype(np.float32)


# revision 21
# speedup vs baseline: 1.0730x; 1.0730x over previous
"""GCN layer on 8 TRN2 NeuronCores (Bass/Tile).

out = segment_sum(edge_vals[:,None] * (X @ W)[edge_col], edge_row, N)

Strategy (1D destination-node sharding, v2):
  - Host: cast/transpose X -> XT bf16 (replicated to all 8 cores). Partition
    edges by destination shard (6250 rows/core) and group by destination
    window (128 rows). One unified gather stream per core (no lo/hi split):
    the XW table packs TWO nodes per 256-byte row (node u in cols 0:64,
    node u+25088 in cols 64:128), so gather indices fit int16 and the
    parity select is done on the consumer side by scaling the two column
    halves with host-precomputed valsE/valsO (val where parity matches,
    else 0) and summing.
  - Device phase 1: XW = X @ W computed redundantly per core (TensorE bf16,
    fp32 PSUM), tiles computed in (lo i, hi i+196) pairs so each 256 B table
    row is written contiguously; stored bf16 to DRAM in partition-major
    order (row r at virtual row (r%128)*196 + r//128) via large DMAs.
  - Device phase 2: dma_gather (SWDGE) fetches table rows per edge; the
    descriptor generation is split round-robin across 4 SWDGE queues so all
    8 GPSIMD cores generate descriptors in parallel (queue q runs on Q7
    cores 2q, 2q+1). VectorE builds S[e, r] = (row_local[e] == r) via an
    iota compare (bf16 meta for 2x DVE rate) and the parity-selected,
    val-scaled rhs; TensorE accumulates S^T @ rhs into the window's PSUM
    [128, 64]. Windows are written out dense - no scatter races anywhere.
  - Host: concatenate the 8 output shards.
"""

from contextlib import ExitStack

import ml_dtypes
import numpy as np

import concourse.bacc as bacc
import concourse.bass as bass
import concourse.mybir as mybir
import concourse.tile as tile
from concourse._compat import get_trn_type
from concourse.bass_utils import run_bass_kernel_spmd

N_NODES = 50000
N_EDGES = 800000
F_IN = 256
F_OUT = 64
N_CORES = 8
SHARD = N_NODES // N_CORES  # 6250 destination rows per core
WIN = 128  # destination rows per PSUM accumulation window
BF16 = ml_dtypes.bfloat16

HALF = 25088  # node-pair split: row r holds node r (cols 0:64) and r+HALF
NTL = HALF // 128  # 196 pair-tiles; table rows = HALF, all int16-addressable

# knobs
SLAB = 3072  # phase-1 node columns per XT slab DMA (per lo/hi stream)
GRP = 28  # phase-1 pair tiles per staged XW store DMA (196 = 7*28)
CH = 16  # phase-2 edge tiles (of 128 edges) per dma_gather call
GB = 16  # phase-2 edge tiles per batched one-hot / rhs build (divides CH)
NQ = 4  # SWDGE queues used round-robin for gather desc-gen
SIM_MEMSET = False  # zero staging tiles (only needed to appease CoreSim)

# test.py pokes these for profiling
TRACE = False
LAST_RESULTS = None


def _install_ntff_hook():
    """The agent image's antenv lacks axon_hooks, so bass_utils' trace=True
    path can't find the NTFF hook. Recreate the module and register the
    ctypes-based hook exactly as trn_agent_boot would."""
    import sys
    import types

    try:
        import antenv.axon_hooks  # noqa: F401

        return True
    except ImportError:
        pass
    try:
        import antenv
        from trn_agent_boot.trn_boot import _ntff_profile_via_ctypes

        mod = types.ModuleType("antenv.axon_hooks")
        mod._hook = None

        def set_axon_ntff_profile_hook(h):
            mod._hook = h

        def get_axon_ntff_profile_hook():
            return mod._hook

        mod.set_axon_ntff_profile_hook = set_axon_ntff_profile_hook
        mod.get_axon_ntff_profile_hook = get_axon_ntff_profile_hook
        sys.modules["antenv.axon_hooks"] = mod
        antenv.axon_hooks = mod
        hook = _ntff_profile_via_ctypes("/opt/axon/libaxon_pjrt.so")
        if hook is not None:
            set_axon_ntff_profile_hook(hook)
        return hook is not None
    except Exception as e:  # profiling is best-effort
        print(f"ntff hook install failed: {e}")
        return False


def _wrap16(stream_i16, n_tiles):
    """Wrapped+replicated dma_gather index layout: stream position i lives at
    partition i%16 (replicated to all 8 16-partition groups), slot i//16."""
    n = n_tiles * 128
    w = np.zeros((128, n // 16), dtype=np.int16)
    s = np.zeros(n, dtype=np.int16)
    s[: len(stream_i16)] = stream_i16
    blk = s.reshape(n // 16, 16).T  # [16, n//16]
    for g in range(8):
        w[g * 16 : (g + 1) * 16, :] = blk
    return w


def _prep(X, W, edge_row, edge_col, edge_vals):
    """Host-side sharding/marshalling.

    Returns (in_maps, T): per-window tile counts (maxed across cores so all
    8 cores run the identical SPMD program).
    """
    XT = np.ascontiguousarray(X.T).astype(BF16)  # [F_IN, N_NODES]
    Wb = np.ascontiguousarray(W).astype(BF16)  # [F_IN, F_OUT]
    # iota_big[p, r*GB + j] = r: a REAL (non-broadcast) operand for the
    # [128, WIN, GB]-layout one-hot build, so both tensor_tensor inputs
    # stream with unit inner stride.
    iota = np.tile(np.repeat(np.arange(WIN, dtype=np.float32), GB), (128, 1))

    n_win = (SHARD + WIN - 1) // WIN  # 49
    core = edge_row // SHARD
    percore = []
    cnt = np.zeros((N_CORES, n_win), dtype=np.int64)
    for p in range(N_CORES):
        m = core == p
        r = edge_row[m].astype(np.int64) - p * SHARD
        c = edge_col[m].astype(np.int64)
        v = edge_vals[m].astype(np.float32)
        w = r // WIN
        tr = c % HALF  # table row
        par = (c >= HALF).astype(np.float32)  # 0 -> cols 0:64, 1 -> 64:128
        q = (tr % 128) * NTL + tr // 128  # partition-major virtual row
        # sort by (window, q): monotone gather addresses within each window
        # give the HBM controller sequential-ish access patterns.
        order = np.lexsort((q, w))
        r, q, v, par, w = r[order], q[order], v[order], par[order], w[order]
        percore.append((r, q, v, par, w))
        cnt[p] = np.bincount(w, minlength=n_win)

    T = np.maximum(1, -(-cnt.max(axis=0) // 128))
    J0 = int(T.sum())
    J = -(-J0 // CH) * CH  # pad to whole gather chunks
    starts = np.concatenate([[0], np.cumsum(T)])

    in_maps = []
    for p in range(N_CORES):
        r, q, v, par, w = percore[p]
        qbuf = np.zeros(J * 128, dtype=np.int64)
        rowloc = np.zeros(J * 128, dtype=np.float32)
        valsE = np.zeros(J * 128, dtype=np.float32)
        valsO = np.zeros(J * 128, dtype=np.float32)
        wstart = np.searchsorted(w, np.arange(n_win))
        wend = np.searchsorted(w, np.arange(n_win), side="right")
        for wi in range(n_win):
            a, b = int(wstart[wi]), int(wend[wi])
            n = b - a
            s0 = int(starts[wi]) * 128
            qbuf[s0 : s0 + n] = q[a:b]
            rowloc[s0 : s0 + n] = (r[a:b] % WIN).astype(np.float32)
            valsE[s0 : s0 + n] = v[a:b] * (1.0 - par[a:b])
            valsO[s0 : s0 + n] = v[a:b] * par[a:b]
        meta = np.concatenate(
            [
                rowloc.reshape(J, 128).T,
                valsE.reshape(J, 128).T,
                valsO.reshape(J, 128).T,
                iota,
            ],
            axis=1,
        ).astype(np.float32)
        in_maps.append(
            {
                "xt": XT,
                "w": Wb,
                "cols": _wrap16(qbuf.astype(np.int16), J),
                "meta": np.ascontiguousarray(meta),
            }
        )
    return in_maps, T


def _build_nc(T, n_nodes=N_NODES, f_in=F_IN, f_out=F_OUT, shard=SHARD):
    f32 = mybir.dt.float32
    bf16 = mybir.dt.bfloat16
    i16 = mybir.dt.int16
    n_win = len(T)
    J0 = int(T.sum())
    J = -(-J0 // CH) * CH
    starts = np.concatenate([[0], np.cumsum(T)])
    n_hi = n_nodes - HALF  # 24912 real hi nodes

    nc = bacc.Bacc(
        get_trn_type() or "TRN2",
        target_bir_lowering=False,
        dynamic_dma_scratch_size=32768,
        num_swdge_queues=NQ,
    )
    xt = nc.dram_tensor("xt", [f_in, n_nodes], bf16, kind="ExternalInput")
    w_in = nc.dram_tensor("w", [f_in, f_out], bf16, kind="ExternalInput")
    cols = nc.dram_tensor("cols", [128, J * 8], i16, kind="ExternalInput")
    meta = nc.dram_tensor("meta", [128, 3 * J + GB * WIN], f32, kind="ExternalInput")
    out = nc.dram_tensor("out", [shard, f_out], f32, kind="ExternalOutput")
    # XW table: virtual row b*NTL + a holds table row r = 128*a + b, which
    # packs node r (cols 0:64) and node r+HALF (cols 64:128) -> 256 B rows.
    xw = nc.dram_tensor("xw", [HALF, 128], bf16, kind="Internal")

    n_kc = f_in // 128  # contraction chunks (2)

    with tile.TileContext(nc) as tc, ExitStack() as ctx:
        const = ctx.enter_context(tc.tile_pool(name="const", bufs=1))
        xt_pool = ctx.enter_context(tc.tile_pool(name="xtp", bufs=2))
        psum1 = ctx.enter_context(tc.tile_pool(name="psum1", bufs=4, space="PSUM"))
        xw_sb = ctx.enter_context(tc.tile_pool(name="xw_sb", bufs=2))
        gath = [
            ctx.enter_context(tc.tile_pool(name=f"gath{qi}", bufs=3))
            for qi in range(NQ)
        ]
        s_pool = ctx.enter_context(tc.tile_pool(name="s_pool", bufs=3))
        rhs_pool = ctx.enter_context(tc.tile_pool(name="rhs_pool", bufs=3))
        tmp_pool = ctx.enter_context(tc.tile_pool(name="tmp_pool", bufs=3))
        psum2 = ctx.enter_context(tc.tile_pool(name="psum2", bufs=4, space="PSUM"))
        out_sb = ctx.enter_context(tc.tile_pool(name="out_sb", bufs=4))

        # resident constants
        w_t = []
        for k in range(n_kc):
            wt = const.tile([128, f_out], bf16, tag=f"w{k}")
            nc.sync.dma_start(out=wt[:], in_=w_in[k * 128 : (k + 1) * 128, :])
            w_t.append(wt)
        meta_t = const.tile([128, 3 * J + GB * WIN], f32, tag="meta")
        nc.sync.dma_start(out=meta_t[:], in_=meta[:, :])
        cols_t = const.tile([128, J * 8], i16, tag="cols")
        nc.sync.dma_start(out=cols_t[:], in_=cols[:, :])

        # shared num_idxs register for all gather calls (one MOVE total)
        nreg = nc.gpsimd.to_reg(CH * 128)

        # ---- phase 1: xw table, computed in (lo i, hi i+NTL) pairs ----
        xw_pm = xw[:, :].rearrange("(b a) f -> b (a f)", b=128)  # [128, NTL*128]
        stg = None
        g0 = 0
        ps = None
        xtl = xth = None
        s_lo = s_hi = 0
        for i in range(NTL):
            if i % (SLAB // 128) == 0:
                s_lo = i * 128
                s_hi = HALF + i * 128
                sl = min(SLAB, HALF - s_lo)
                sh = min(SLAB, n_nodes - s_hi)
                xtl, xth = [], []
                for k in range(n_kc):
                    t1 = xt_pool.tile([128, SLAB], bf16, tag=f"xtl{k}")
                    nc.sync.dma_start(
                        out=t1[:, :sl], in_=xt[k * 128 : (k + 1) * 128, s_lo : s_lo + sl]
                    )
                    xtl.append(t1)
                    if sh > 0:
                        t2 = xt_pool.tile([128, SLAB], bf16, tag=f"xth{k}")
                        nc.sync.dma_start(
                            out=t2[:, :sh],
                            in_=xt[k * 128 : (k + 1) * 128, s_hi : s_hi + sh],
                        )
                        xth.append(t2)
            if i % GRP == 0:
                g0 = i
                stg = xw_sb.tile([128, GRP * 128], bf16, tag="stg")
                if SIM_MEMSET:
                    nc.gpsimd.memset(stg[:], 0)
            if i % 2 == 0:
                ps = psum1.tile([128, 256], f32, tag="ps1")
            off = (i % 2) * 128
            lo0 = i * 128 - s_lo
            for k in range(n_kc):
                nc.tensor.matmul(
                    out=ps[:, off : off + f_out],
                    lhsT=xtl[k][:, lo0 : lo0 + 128],
                    rhs=w_t[k][:],
                    start=(k == 0),
                    stop=(k == n_kc - 1),
                )
            m_hi = min(128, n_hi - i * 128)
            if m_hi > 0:
                hi0 = (HALF + i * 128) - s_hi
                for k in range(n_kc):
                    nc.tensor.matmul(
                        out=ps[:m_hi, off + f_out : off + 128],
                        lhsT=xth[k][:, hi0 : hi0 + m_hi],
                        rhs=w_t[k][:],
                        start=(k == 0),
                        stop=(k == n_kc - 1),
                    )
            if i % 2 == 1 or i == NTL - 1:
                loc = ((i - 1 if i % 2 == 1 else i) - g0) * 128
                ncols = 256 if i % 2 == 1 else 128
                dst = stg[:, loc : loc + ncols]
                if (i // 2) % 2 == 0:
                    nc.scalar.activation(
                        out=dst, in_=ps[:, :ncols],
                        func=mybir.ActivationFunctionType.Copy,
                    )
                else:
                    nc.vector.tensor_copy(out=dst, in_=ps[:, :ncols])
            if i == NTL - 1 or (i + 1) % GRP == 0:
                gn = i + 1 - g0
                nc.sync.dma_start(
                    out=xw_pm[:, g0 * 128 : (g0 + gn) * 128], in_=stg[:, : gn * 128]
                )

        # ---- phase 2: multi-queue dma_gather + one-hot matmul segment-sum ----
        chunks = {}
        batches = {}

        def ensure_chunk(tile_idx):
            ci = tile_idx // CH
            if ci in chunks:
                return chunks[ci]
            q = ci % NQ
            g = gath[q].tile([128, CH, 128], bf16, tag=f"g{q}")
            nc.gpsimd.dma_gather(
                out_ap=g[:, :, :],
                in_ap=xw[:, :],
                idxs_ap=cols_t[:, ci * CH * 8 : (ci + 1) * CH * 8],
                num_idxs=CH * 128,
                num_idxs_reg=nreg,
                elem_size=128,
                single_packet=False,
                queue_num=q,
            )
            chunks[ci] = g
            return g

        def ensure_batch(tile_idx):
            bi = tile_idx // GB
            if bi in batches:
                return batches[bi]
            b0 = bi * GB
            g = ensure_chunk(b0)
            gs = b0 - (b0 // CH) * CH
            S_b = s_pool.tile([128, WIN, GB], bf16, tag="S")
            rhs_b = rhs_pool.tile([128, GB, f_out], bf16, tag="rhs")
            tmp_b = tmp_pool.tile([128, GB, f_out], bf16, tag="tmp")
            nc.vector.tensor_tensor(
                out=S_b[:],
                in0=meta_t[:, 3 * J : 3 * J + WIN * GB].rearrange(
                    "p (r b) -> p r b", b=GB
                ),
                in1=meta_t[:, b0 : b0 + GB]
                .rearrange("p (one b) -> p one b", one=1)
                .to_broadcast([128, WIN, GB]),
                op=mybir.AluOpType.is_equal,
            )
            nc.vector.tensor_tensor(
                out=rhs_b[:],
                in0=g[:, gs : gs + GB, 0:f_out],
                in1=meta_t[:, J + b0 : J + b0 + GB].to_broadcast([128, GB, f_out]),
                op=mybir.AluOpType.mult,
            )
            nc.vector.tensor_tensor(
                out=tmp_b[:],
                in0=g[:, gs : gs + GB, f_out:128],
                in1=meta_t[:, 2 * J + b0 : 2 * J + b0 + GB].to_broadcast(
                    [128, GB, f_out]
                ),
                op=mybir.AluOpType.mult,
            )
            batches[bi] = (S_b, rhs_b, tmp_b)
            return batches[bi]

        for w in range(n_win):
            cur_ps = psum2.tile([128, f_out], f32, tag="ps2")
            n_t = int(T[w])
            for k in range(n_t):
                t_s = int(starts[w]) + k
                S_b, rhs_b, tmp_b = ensure_batch(t_s)
                sl = t_s % GB
                lhsT = S_b[:, :, sl : sl + 1].rearrange("p r one -> p (r one)")
                nc.tensor.matmul(
                    out=cur_ps[:],
                    lhsT=lhsT,
                    rhs=rhs_b[:, sl : sl + 1, :],
                    start=(k == 0),
                    stop=False,
                )
                nc.tensor.matmul(
                    out=cur_ps[:],
                    lhsT=lhsT,
                    rhs=tmp_b[:, sl : sl + 1, :],
                    start=False,
                    stop=(k == n_t - 1),
                )
            rows = min(WIN, shard - w * WIN)
            ot = out_sb.tile([128, f_out], f32, tag="ot")
            nc.scalar.activation(
                out=ot[:rows, :],
                in_=cur_ps[:rows, :],
                func=mybir.ActivationFunctionType.Copy,
            )
            nc.sync.dma_start(out=out[w * WIN : w * WIN + rows, :], in_=ot[:rows, :])
    nc.compile()
    return nc


def kernel(X, W, edge_row, edge_col, edge_vals):
    global LAST_RESULTS
    X = np.asarray(X, dtype=np.float32)
    W = np.asarray(W, dtype=np.float32)
    edge_row = np.asarray(edge_row, dtype=np.int32)
    edge_col = np.asarray(edge_col, dtype=np.int32)
    edge_vals = np.asarray(edge_vals, dtype=np.float32)

    in_maps, T = _prep(X, W, edge_row, edge_col, edge_vals)
    nc = _build_nc(T)
    trace = TRACE and _install_ntff_hook()
    res = run_bass_kernel_spmd(
        nc, in_maps, core_ids=list(range(N_CORES)), trace=trace
    )
    LAST_RESULTS = res
    out = np.concatenate([res.results[p]["out"] for p in range(N_CORES)], axis=0)
    return out.astype(np.float32)


# revision 24
# speedup vs baseline: 1.1997x; 1.1180x over previous
"""GCN layer on 8 TRN2 NeuronCores (Bass/Tile).

out = segment_sum(edge_vals[:,None] * (X @ W)[edge_col], edge_row, N)

Strategy (1D destination-node sharding, v2):
  - Host: cast/transpose X -> XT bf16 (replicated to all 8 cores). Partition
    edges by destination shard (6250 rows/core) and group by destination
    window (128 rows). One unified gather stream per core (no lo/hi split):
    the XW table packs TWO nodes per 256-byte row (node u in cols 0:64,
    node u+25088 in cols 64:128), so gather indices fit int16 and the
    parity select is done on the consumer side by scaling the two column
    halves with host-precomputed valsE/valsO (val where parity matches,
    else 0) and summing.
  - Device phase 1: XW = X @ W computed redundantly per core (TensorE bf16,
    fp32 PSUM), tiles computed in (lo i, hi i+196) pairs so each 256 B table
    row is written contiguously; stored bf16 to DRAM in partition-major
    order (row r at virtual row (r%128)*196 + r//128) via large DMAs.
  - Device phase 2: dma_gather (SWDGE) fetches table rows per edge; the
    descriptor generation is split round-robin across 4 SWDGE queues so all
    8 GPSIMD cores generate descriptors in parallel (queue q runs on Q7
    cores 2q, 2q+1). VectorE builds S[e, r] = (row_local[e] == r) via an
    iota compare (bf16 meta for 2x DVE rate) and the parity-selected,
    val-scaled rhs; TensorE accumulates S^T @ rhs into the window's PSUM
    [128, 64]. Windows are written out dense - no scatter races anywhere.
  - Host: concatenate the 8 output shards.
"""

from contextlib import ExitStack

import ml_dtypes
import numpy as np

import concourse.bacc as bacc
import concourse.bass as bass
import concourse.mybir as mybir
import concourse.tile as tile
from concourse._compat import get_trn_type
from concourse.bass_utils import run_bass_kernel_spmd

N_NODES = 50000
N_EDGES = 800000
F_IN = 256
F_OUT = 64
N_CORES = 8
SHARD = N_NODES // N_CORES  # 6250 destination rows per core
WIN = 128  # destination rows per PSUM accumulation window
BF16 = ml_dtypes.bfloat16

HALF = 25088  # node-pair split: row r holds node r (cols 0:64) and r+HALF
NTL = HALF // 128  # 196 pair-tiles; table rows = HALF, all int16-addressable

# knobs
SLAB = 3072  # phase-1 node columns per XT slab DMA (per lo/hi stream)
GRP = 28  # phase-1 pair tiles per staged XW store DMA (196 = 7*28)
CH = 16  # phase-2 edge tiles (of 128 edges) per dma_gather call
GB = 16  # phase-2 edge tiles per batched one-hot / rhs build (divides CH)
NQ = 4  # SWDGE queues used round-robin for gather desc-gen
SIM_MEMSET = False  # zero staging tiles (only needed to appease CoreSim)

# test.py pokes these for profiling
TRACE = False
LAST_RESULTS = None


def _install_ntff_hook():
    """The agent image's antenv lacks axon_hooks, so bass_utils' trace=True
    path can't find the NTFF hook. Recreate the module and register the
    ctypes-based hook exactly as trn_agent_boot would."""
    import sys
    import types

    try:
        import antenv.axon_hooks  # noqa: F401

        return True
    except ImportError:
        pass
    try:
        import antenv
        from trn_agent_boot.trn_boot import _ntff_profile_via_ctypes

        mod = types.ModuleType("antenv.axon_hooks")
        mod._hook = None

        def set_axon_ntff_profile_hook(h):
            mod._hook = h

        def get_axon_ntff_profile_hook():
            return mod._hook

        mod.set_axon_ntff_profile_hook = set_axon_ntff_profile_hook
        mod.get_axon_ntff_profile_hook = get_axon_ntff_profile_hook
        sys.modules["antenv.axon_hooks"] = mod
        antenv.axon_hooks = mod
        hook = _ntff_profile_via_ctypes("/opt/axon/libaxon_pjrt.so")
        if hook is not None:
            set_axon_ntff_profile_hook(hook)
        return hook is not None
    except Exception as e:  # profiling is best-effort
        print(f"ntff hook install failed: {e}")
        return False


def _wrap16(stream_i16, n_tiles):
    """Wrapped+replicated dma_gather index layout: stream position i lives at
    partition i%16 (replicated to all 8 16-partition groups), slot i//16."""
    n = n_tiles * 128
    w = np.zeros((128, n // 16), dtype=np.int16)
    s = np.zeros(n, dtype=np.int16)
    s[: len(stream_i16)] = stream_i16
    blk = s.reshape(n // 16, 16).T  # [16, n//16]
    for g in range(8):
        w[g * 16 : (g + 1) * 16, :] = blk
    return w


def _prep(X, W, edge_row, edge_col, edge_vals):
    """Host-side sharding/marshalling.

    Returns (in_maps, T): per-window tile counts (maxed across cores so all
    8 cores run the identical SPMD program).
    """
    XT = np.ascontiguousarray(X.T).astype(BF16)  # [F_IN, N_NODES]
    Wb = np.ascontiguousarray(W).astype(BF16)  # [F_IN, F_OUT]
    # iota_big[p, r*GB + j] = r: a REAL (non-broadcast) operand for the
    # [128, WIN, GB]-layout one-hot build, so both tensor_tensor inputs
    # stream with unit inner stride.
    iota = np.tile(np.repeat(np.arange(WIN, dtype=np.float32), GB), (128, 1))

    n_win = (SHARD + WIN - 1) // WIN  # 49
    core = edge_row // SHARD
    percore = []
    cnt = np.zeros((N_CORES, n_win), dtype=np.int64)
    for p in range(N_CORES):
        m = core == p
        r = edge_row[m].astype(np.int64) - p * SHARD
        c = edge_col[m].astype(np.int64)
        v = edge_vals[m].astype(np.float32)
        w = r // WIN
        tr = c % HALF  # table row
        par = (c >= HALF).astype(np.float32)  # 0 -> cols 0:64, 1 -> 64:128
        q = (tr % 128) * NTL + tr // 128  # partition-major virtual row
        # sort by (window, q): monotone gather addresses within each window
        # give the HBM controller sequential-ish access patterns.
        order = np.lexsort((q, w))
        r, q, v, par, w = r[order], q[order], v[order], par[order], w[order]
        percore.append((r, q, v, par, w))
        cnt[p] = np.bincount(w, minlength=n_win)

    T = np.maximum(1, -(-cnt.max(axis=0) // 128))
    J0 = int(T.sum())
    J = -(-J0 // CH) * CH  # pad to whole gather chunks
    starts = np.concatenate([[0], np.cumsum(T)])

    in_maps = []
    for p in range(N_CORES):
        r, q, v, par, w = percore[p]
        qbuf = np.zeros(J * 128, dtype=np.int64)
        rowloc = np.zeros(J * 128, dtype=np.float32)
        valsE = np.zeros(J * 128, dtype=np.float32)
        valsO = np.zeros(J * 128, dtype=np.float32)
        wstart = np.searchsorted(w, np.arange(n_win))
        wend = np.searchsorted(w, np.arange(n_win), side="right")
        for wi in range(n_win):
            a, b = int(wstart[wi]), int(wend[wi])
            n = b - a
            s0 = int(starts[wi]) * 128
            qbuf[s0 : s0 + n] = q[a:b]
            rowloc[s0 : s0 + n] = (r[a:b] % WIN).astype(np.float32)
            valsE[s0 : s0 + n] = v[a:b] * (1.0 - par[a:b])
            valsO[s0 : s0 + n] = v[a:b] * par[a:b]
        meta = np.concatenate(
            [
                rowloc.reshape(J, 128).T,
                valsE.reshape(J, 128).T,
                valsO.reshape(J, 128).T,
                iota,
            ],
            axis=1,
        ).astype(np.float32)
        in_maps.append(
            {
                "xt": XT,
                "w": Wb,
                "cols": _wrap16(qbuf.astype(np.int16), J),
                "meta": np.ascontiguousarray(meta),
            }
        )
    return in_maps, T


def _build_nc(T, n_nodes=N_NODES, f_in=F_IN, f_out=F_OUT, shard=SHARD):
    f32 = mybir.dt.float32
    bf16 = mybir.dt.bfloat16
    i16 = mybir.dt.int16
    n_win = len(T)
    J0 = int(T.sum())
    J = -(-J0 // CH) * CH
    starts = np.concatenate([[0], np.cumsum(T)])
    n_hi = n_nodes - HALF  # 24912 real hi nodes

    nc = bacc.Bacc(
        get_trn_type() or "TRN2",
        target_bir_lowering=False,
        dynamic_dma_scratch_size=32768,
        num_swdge_queues=NQ,
    )
    xt = nc.dram_tensor("xt", [f_in, n_nodes], bf16, kind="ExternalInput")
    w_in = nc.dram_tensor("w", [f_in, f_out], bf16, kind="ExternalInput")
    cols = nc.dram_tensor("cols", [128, J * 8], i16, kind="ExternalInput")
    meta = nc.dram_tensor("meta", [128, 3 * J + GB * WIN], f32, kind="ExternalInput")
    out = nc.dram_tensor("out", [shard, f_out], f32, kind="ExternalOutput")
    # XW table: virtual row b*NTL + a holds table row r = 128*a + b, which
    # packs node r (cols 0:64) and node r+HALF (cols 64:128) -> 256 B rows.
    xw = nc.dram_tensor("xw", [HALF, 128], bf16, kind="Internal")

    n_kc = f_in // 128  # contraction chunks (2)

    with tile.TileContext(nc) as tc, ExitStack() as ctx:
        const = ctx.enter_context(tc.tile_pool(name="const", bufs=1))
        xt_pool = ctx.enter_context(tc.tile_pool(name="xtp", bufs=3))
        psum1 = ctx.enter_context(tc.tile_pool(name="psum1", bufs=4, space="PSUM"))
        xw_sb = ctx.enter_context(tc.tile_pool(name="xw_sb", bufs=2))
        gath = [
            ctx.enter_context(tc.tile_pool(name=f"gath{qi}", bufs=3))
            for qi in range(NQ)
        ]
        s_pool = ctx.enter_context(tc.tile_pool(name="s_pool", bufs=3))
        rhs_pool = ctx.enter_context(tc.tile_pool(name="rhs_pool", bufs=3))
        tmp_pool = ctx.enter_context(tc.tile_pool(name="tmp_pool", bufs=3))
        psum2 = ctx.enter_context(tc.tile_pool(name="psum2", bufs=4, space="PSUM"))
        out_sb = ctx.enter_context(tc.tile_pool(name="out_sb", bufs=4))

        # resident constants
        w_t = []
        for k in range(n_kc):
            wt = const.tile([128, f_out], bf16, tag=f"w{k}")
            nc.sync.dma_start(out=wt[:], in_=w_in[k * 128 : (k + 1) * 128, :])
            w_t.append(wt)
        meta_t = const.tile([128, 3 * J + GB * WIN], f32, tag="meta")
        nc.sync.dma_start(out=meta_t[:], in_=meta[:, :])
        cols_t = const.tile([128, J * 8], i16, tag="cols")
        nc.sync.dma_start(out=cols_t[:], in_=cols[:, :])

        # shared num_idxs register for all gather calls (one MOVE total)
        nreg = nc.gpsimd.to_reg(CH * 128)

        # ---- phase 1: xw table, computed in (lo i, hi i+NTL) pairs ----
        xw_pm = xw[:, :].rearrange("(b a) f -> b (a f)", b=128)  # [128, NTL*128]
        stg = None
        g0 = 0
        ps = None
        xtl = xth = None
        s_lo = s_hi = 0
        for i in range(NTL):
            if i % (SLAB // 128) == 0:
                s_lo = i * 128
                s_hi = HALF + i * 128
                sl = min(SLAB, HALF - s_lo)
                sh = min(SLAB, n_nodes - s_hi)
                xtl, xth = [], []
                for k in range(n_kc):
                    t1 = xt_pool.tile([128, SLAB], bf16, tag=f"xtl{k}")
                    nc.sync.dma_start(
                        out=t1[:, :sl], in_=xt[k * 128 : (k + 1) * 128, s_lo : s_lo + sl]
                    )
                    xtl.append(t1)
                    if sh > 0:
                        t2 = xt_pool.tile([128, SLAB], bf16, tag=f"xth{k}")
                        nc.sync.dma_start(
                            out=t2[:, :sh],
                            in_=xt[k * 128 : (k + 1) * 128, s_hi : s_hi + sh],
                        )
                        xth.append(t2)
            if i % GRP == 0:
                g0 = i
                stg = xw_sb.tile([128, GRP * 128], bf16, tag="stg")
                if SIM_MEMSET:
                    nc.gpsimd.memset(stg[:], 0)
            if i % 2 == 0:
                ps = psum1.tile([128, 256], f32, tag="ps1")
            off = (i % 2) * 128
            lo0 = i * 128 - s_lo
            for k in range(n_kc):
                nc.tensor.matmul(
                    out=ps[:, off : off + f_out],
                    lhsT=xtl[k][:, lo0 : lo0 + 128],
                    rhs=w_t[k][:],
                    start=(k == 0),
                    stop=(k == n_kc - 1),
                )
            m_hi = min(128, n_hi - i * 128)
            if m_hi > 0:
                hi0 = (HALF + i * 128) - s_hi
                for k in range(n_kc):
                    nc.tensor.matmul(
                        out=ps[:m_hi, off + f_out : off + 128],
                        lhsT=xth[k][:, hi0 : hi0 + m_hi],
                        rhs=w_t[k][:],
                        start=(k == 0),
                        stop=(k == n_kc - 1),
                    )
            if i % 2 == 1 or i == NTL - 1:
                loc = ((i - 1 if i % 2 == 1 else i) - g0) * 128
                ncols = 256 if i % 2 == 1 else 128
                dst = stg[:, loc : loc + ncols]
                if (i // 2) % 2 == 0:
                    nc.scalar.activation(
                        out=dst, in_=ps[:, :ncols],
                        func=mybir.ActivationFunctionType.Copy,
                    )
                else:
                    nc.vector.tensor_copy(out=dst, in_=ps[:, :ncols])
            if i == NTL - 1 or (i + 1) % GRP == 0:
                gn = i + 1 - g0
                nc.sync.dma_start(
                    out=xw_pm[:, g0 * 128 : (g0 + gn) * 128], in_=stg[:, : gn * 128]
                )

        # ---- phase 2: multi-queue dma_gather + one-hot matmul segment-sum ----
        chunks = {}
        batches = {}

        def ensure_chunk(tile_idx):
            ci = tile_idx // CH
            if ci in chunks:
                return chunks[ci]
            q = ci % NQ
            g = gath[q].tile([128, CH, 128], bf16, tag=f"g{q}")
            nc.gpsimd.dma_gather(
                out_ap=g[:, :, :],
                in_ap=xw[:, :],
                idxs_ap=cols_t[:, ci * CH * 8 : (ci + 1) * CH * 8],
                num_idxs=CH * 128,
                num_idxs_reg=nreg,
                elem_size=128,
                single_packet=False,
                queue_num=q,
            )
            chunks[ci] = g
            return g

        def ensure_batch(tile_idx):
            bi = tile_idx // GB
            if bi in batches:
                return batches[bi]
            b0 = bi * GB
            g = ensure_chunk(b0)
            gs = b0 - (b0 // CH) * CH
            S_b = s_pool.tile([128, WIN, GB], bf16, tag="S")
            rhs_b = rhs_pool.tile([128, GB, f_out], bf16, tag="rhs")
            tmp_b = tmp_pool.tile([128, GB, f_out], bf16, tag="tmp")
            nc.vector.tensor_tensor(
                out=S_b[:],
                in0=meta_t[:, 3 * J : 3 * J + WIN * GB].rearrange(
                    "p (r b) -> p r b", b=GB
                ),
                in1=meta_t[:, b0 : b0 + GB]
                .rearrange("p (one b) -> p one b", one=1)
                .to_broadcast([128, WIN, GB]),
                op=mybir.AluOpType.is_equal,
            )
            nc.vector.tensor_tensor(
                out=rhs_b[:],
                in0=g[:, gs : gs + GB, 0:f_out],
                in1=meta_t[:, J + b0 : J + b0 + GB].to_broadcast([128, GB, f_out]),
                op=mybir.AluOpType.mult,
            )
            nc.vector.tensor_tensor(
                out=tmp_b[:],
                in0=g[:, gs : gs + GB, f_out:128],
                in1=meta_t[:, 2 * J + b0 : 2 * J + b0 + GB].to_broadcast(
                    [128, GB, f_out]
                ),
                op=mybir.AluOpType.mult,
            )
            batches[bi] = (S_b, rhs_b, tmp_b)
            return batches[bi]

        for w in range(n_win):
            cur_ps = psum2.tile([128, f_out], f32, tag="ps2")
            n_t = int(T[w])
            for k in range(n_t):
                t_s = int(starts[w]) + k
                S_b, rhs_b, tmp_b = ensure_batch(t_s)
                sl = t_s % GB
                lhsT = S_b[:, :, sl : sl + 1].rearrange("p r one -> p (r one)")
                nc.tensor.matmul(
                    out=cur_ps[:],
                    lhsT=lhsT,
                    rhs=rhs_b[:, sl : sl + 1, :],
                    start=(k == 0),
                    stop=False,
                )
                nc.tensor.matmul(
                    out=cur_ps[:],
                    lhsT=lhsT,
                    rhs=tmp_b[:, sl : sl + 1, :],
                    start=False,
                    stop=(k == n_t - 1),
                )
            rows = min(WIN, shard - w * WIN)
            ot = out_sb.tile([128, f_out], f32, tag="ot")
            nc.vector.tensor_copy(out=ot[:rows, :], in_=cur_ps[:rows, :])
            nc.sync.dma_start(out=out[w * WIN : w * WIN + rows, :], in_=ot[:rows, :])
    nc.compile()
    return nc


def kernel(X, W, edge_row, edge_col, edge_vals):
    global LAST_RESULTS
    X = np.asarray(X, dtype=np.float32)
    W = np.asarray(W, dtype=np.float32)
    edge_row = np.asarray(edge_row, dtype=np.int32)
    edge_col = np.asarray(edge_col, dtype=np.int32)
    edge_vals = np.asarray(edge_vals, dtype=np.float32)

    in_maps, T = _prep(X, W, edge_row, edge_col, edge_vals)
    nc = _build_nc(T)
    trace = TRACE and _install_ntff_hook()
    res = run_bass_kernel_spmd(
        nc, in_maps, core_ids=list(range(N_CORES)), trace=trace
    )
    LAST_RESULTS = res
    out = np.concatenate([res.results[p]["out"] for p in range(N_CORES)], axis=0)
    return out.astype(np.float32)


# revision 27
# speedup vs baseline: 1.2153x; 1.0130x over previous
"""GCN layer on 8 TRN2 NeuronCores (Bass/Tile).

out = segment_sum(edge_vals[:,None] * (X @ W)[edge_col], edge_row, N)

Strategy (1D destination-node sharding, v2):
  - Host: cast/transpose X -> XT bf16 (replicated to all 8 cores). Partition
    edges by destination shard (6250 rows/core) and group by destination
    window (128 rows). One unified gather stream per core (no lo/hi split):
    the XW table packs TWO nodes per 256-byte row (node u in cols 0:64,
    node u+25088 in cols 64:128), so gather indices fit int16 and the
    parity select is done on the consumer side by scaling the two column
    halves with host-precomputed valsE/valsO (val where parity matches,
    else 0) and summing.
  - Device phase 1: XW = X @ W computed redundantly per core (TensorE bf16,
    fp32 PSUM), tiles computed in (lo i, hi i+196) pairs so each 256 B table
    row is written contiguously; stored bf16 to DRAM in partition-major
    order (row r at virtual row (r%128)*196 + r//128) via large DMAs.
  - Device phase 2: dma_gather (SWDGE) fetches table rows per edge; the
    descriptor generation is split round-robin across 4 SWDGE queues so all
    8 GPSIMD cores generate descriptors in parallel (queue q runs on Q7
    cores 2q, 2q+1). VectorE builds S[e, r] = (row_local[e] == r) via an
    iota compare (bf16 meta for 2x DVE rate) and the parity-selected,
    val-scaled rhs; TensorE accumulates S^T @ rhs into the window's PSUM
    [128, 64]. Windows are written out dense - no scatter races anywhere.
  - Host: concatenate the 8 output shards.
"""

from contextlib import ExitStack

import ml_dtypes
import numpy as np

import concourse.bacc as bacc
import concourse.bass as bass
import concourse.mybir as mybir
import concourse.tile as tile
from concourse._compat import get_trn_type
from concourse.bass_utils import run_bass_kernel_spmd

N_NODES = 50000
N_EDGES = 800000
F_IN = 256
F_OUT = 64
N_CORES = 8
SHARD = N_NODES // N_CORES  # 6250 destination rows per core
WIN = 128  # destination rows per PSUM accumulation window
BF16 = ml_dtypes.bfloat16

HALF = 25088  # node-pair split: row r holds node r (cols 0:64) and r+HALF
NTL = HALF // 128  # 196 pair-tiles; table rows = HALF, all int16-addressable

# knobs
SLAB = 2048  # phase-1 node columns per XT slab DMA (per lo/hi stream)
GRP = 28  # phase-1 pair tiles per staged XW store DMA (196 = 7*28)
CH = 16  # phase-2 edge tiles (of 128 edges) per dma_gather call
GB = 16  # phase-2 edge tiles per batched one-hot / rhs build (divides CH)
NQ = 4  # SWDGE queues used round-robin for gather desc-gen
SIM_MEMSET = False  # zero staging tiles (only needed to appease CoreSim)

# test.py pokes these for profiling
TRACE = False
LAST_RESULTS = None


def _install_ntff_hook():
    """The agent image's antenv lacks axon_hooks, so bass_utils' trace=True
    path can't find the NTFF hook. Recreate the module and register the
    ctypes-based hook exactly as trn_agent_boot would."""
    import sys
    import types

    try:
        import antenv.axon_hooks  # noqa: F401

        return True
    except ImportError:
        pass
    try:
        import antenv
        from trn_agent_boot.trn_boot import _ntff_profile_via_ctypes

        mod = types.ModuleType("antenv.axon_hooks")
        mod._hook = None

        def set_axon_ntff_profile_hook(h):
            mod._hook = h

        def get_axon_ntff_profile_hook():
            return mod._hook

        mod.set_axon_ntff_profile_hook = set_axon_ntff_profile_hook
        mod.get_axon_ntff_profile_hook = get_axon_ntff_profile_hook
        sys.modules["antenv.axon_hooks"] = mod
        antenv.axon_hooks = mod
        hook = _ntff_profile_via_ctypes("/opt/axon/libaxon_pjrt.so")
        if hook is not None:
            set_axon_ntff_profile_hook(hook)
        return hook is not None
    except Exception as e:  # profiling is best-effort
        print(f"ntff hook install failed: {e}")
        return False


def _wrap16(stream_i16, n_tiles):
    """Wrapped+replicated dma_gather index layout: stream position i lives at
    partition i%16 (replicated to all 8 16-partition groups), slot i//16."""
    n = n_tiles * 128
    w = np.zeros((128, n // 16), dtype=np.int16)
    s = np.zeros(n, dtype=np.int16)
    s[: len(stream_i16)] = stream_i16
    blk = s.reshape(n // 16, 16).T  # [16, n//16]
    for g in range(8):
        w[g * 16 : (g + 1) * 16, :] = blk
    return w


def _prep(X, W, edge_row, edge_col, edge_vals):
    """Host-side sharding/marshalling.

    Returns (in_maps, T): per-window tile counts (maxed across cores so all
    8 cores run the identical SPMD program).
    """
    XT = np.ascontiguousarray(X.T).astype(BF16)  # [F_IN, N_NODES]
    Wb = np.ascontiguousarray(W).astype(BF16)  # [F_IN, F_OUT]
    # iota_big[p, r*GB + j] = r: a REAL (non-broadcast) operand for the
    # [128, WIN, GB]-layout one-hot build, so both tensor_tensor inputs
    # stream with unit inner stride.
    iota = np.tile(np.repeat(np.arange(WIN, dtype=np.float32), GB), (128, 1))

    n_win = (SHARD + WIN - 1) // WIN  # 49
    core = edge_row // SHARD
    percore = []
    cnt = np.zeros((N_CORES, n_win), dtype=np.int64)
    for p in range(N_CORES):
        m = core == p
        r = edge_row[m].astype(np.int64) - p * SHARD
        c = edge_col[m].astype(np.int64)
        v = edge_vals[m].astype(np.float32)
        w = r // WIN
        tr = c % HALF  # table row
        par = (c >= HALF).astype(np.float32)  # 0 -> cols 0:64, 1 -> 64:128
        q = (tr % 128) * NTL + tr // 128  # partition-major virtual row
        # sort by (window, q): monotone gather addresses within each window
        # give the HBM controller sequential-ish access patterns.
        order = np.lexsort((q, w))
        r, q, v, par, w = r[order], q[order], v[order], par[order], w[order]
        percore.append((r, q, v, par, w))
        cnt[p] = np.bincount(w, minlength=n_win)

    T = np.maximum(1, -(-cnt.max(axis=0) // 128))
    J0 = int(T.sum())
    J = -(-J0 // CH) * CH  # pad to whole gather chunks
    starts = np.concatenate([[0], np.cumsum(T)])

    in_maps = []
    for p in range(N_CORES):
        r, q, v, par, w = percore[p]
        qbuf = np.zeros(J * 128, dtype=np.int64)
        rowloc = np.zeros(J * 128, dtype=np.float32)
        valsE = np.zeros(J * 128, dtype=np.float32)
        valsO = np.zeros(J * 128, dtype=np.float32)
        wstart = np.searchsorted(w, np.arange(n_win))
        wend = np.searchsorted(w, np.arange(n_win), side="right")
        for wi in range(n_win):
            a, b = int(wstart[wi]), int(wend[wi])
            n = b - a
            s0 = int(starts[wi]) * 128
            qbuf[s0 : s0 + n] = q[a:b]
            rowloc[s0 : s0 + n] = (r[a:b] % WIN).astype(np.float32)
            valsE[s0 : s0 + n] = v[a:b] * (1.0 - par[a:b])
            valsO[s0 : s0 + n] = v[a:b] * par[a:b]
        meta = np.concatenate(
            [
                rowloc.reshape(J, 128).T,
                valsE.reshape(J, 128).T,
                valsO.reshape(J, 128).T,
                iota,
            ],
            axis=1,
        ).astype(np.float32)
        in_maps.append(
            {
                "xt": XT,
                "w": Wb,
                "cols": _wrap16(qbuf.astype(np.int16), J),
                "meta": np.ascontiguousarray(meta),
            }
        )
    return in_maps, T


def _build_nc(T, n_nodes=N_NODES, f_in=F_IN, f_out=F_OUT, shard=SHARD):
    f32 = mybir.dt.float32
    bf16 = mybir.dt.bfloat16
    i16 = mybir.dt.int16
    n_win = len(T)
    J0 = int(T.sum())
    J = -(-J0 // CH) * CH
    starts = np.concatenate([[0], np.cumsum(T)])
    n_hi = n_nodes - HALF  # 24912 real hi nodes

    nc = bacc.Bacc(
        get_trn_type() or "TRN2",
        target_bir_lowering=False,
        dynamic_dma_scratch_size=32768,
        num_swdge_queues=NQ,
    )
    xt = nc.dram_tensor("xt", [f_in, n_nodes], bf16, kind="ExternalInput")
    w_in = nc.dram_tensor("w", [f_in, f_out], bf16, kind="ExternalInput")
    cols = nc.dram_tensor("cols", [128, J * 8], i16, kind="ExternalInput")
    meta = nc.dram_tensor("meta", [128, 3 * J + GB * WIN], f32, kind="ExternalInput")
    out = nc.dram_tensor("out", [shard, f_out], f32, kind="ExternalOutput")
    # XW table: virtual row b*NTL + a holds table row r = 128*a + b, which
    # packs node r (cols 0:64) and node r+HALF (cols 64:128) -> 256 B rows.
    xw = nc.dram_tensor("xw", [HALF, 128], bf16, kind="Internal")

    n_kc = f_in // 128  # contraction chunks (2)

    with tile.TileContext(nc) as tc, ExitStack() as ctx:
        const = ctx.enter_context(tc.tile_pool(name="const", bufs=1))
        xt_pool = ctx.enter_context(tc.tile_pool(name="xtp", bufs=2))
        psum1 = ctx.enter_context(tc.tile_pool(name="psum1", bufs=4, space="PSUM"))
        xw_sb = ctx.enter_context(tc.tile_pool(name="xw_sb", bufs=2))
        gath = [
            ctx.enter_context(tc.tile_pool(name=f"gath{qi}", bufs=3))
            for qi in range(NQ)
        ]
        s_pool = ctx.enter_context(tc.tile_pool(name="s_pool", bufs=3))
        rhs_pool = ctx.enter_context(tc.tile_pool(name="rhs_pool", bufs=3))
        tmp_pool = ctx.enter_context(tc.tile_pool(name="tmp_pool", bufs=3))
        psum2 = ctx.enter_context(tc.tile_pool(name="psum2", bufs=4, space="PSUM"))
        out_sb = ctx.enter_context(tc.tile_pool(name="out_sb", bufs=4))

        # resident constants
        w_t = []
        for k in range(n_kc):
            wt = const.tile([128, f_out], bf16, tag=f"w{k}")
            nc.sync.dma_start(out=wt[:], in_=w_in[k * 128 : (k + 1) * 128, :])
            w_t.append(wt)
        meta_t = const.tile([128, 3 * J + GB * WIN], f32, tag="meta")
        nc.sync.dma_start(out=meta_t[:], in_=meta[:, :])
        cols_t = const.tile([128, J * 8], i16, tag="cols")
        nc.sync.dma_start(out=cols_t[:], in_=cols[:, :])

        # shared num_idxs register for all gather calls (one MOVE total)
        nreg = nc.gpsimd.to_reg(CH * 128)

        # ---- phase 1: xw table, computed in (lo i, hi i+NTL) pairs ----
        xw_pm = xw[:, :].rearrange("(b a) f -> b (a f)", b=128)  # [128, NTL*128]
        stg = None
        g0 = 0
        ps = None
        xtl = xth = None
        s_lo = s_hi = 0
        for i in range(NTL):
            if i % (SLAB // 128) == 0:
                s_lo = i * 128
                s_hi = HALF + i * 128
                sl = min(SLAB, HALF - s_lo)
                sh = min(SLAB, n_nodes - s_hi)
                xtl, xth = [], []
                for k in range(n_kc):
                    t1 = xt_pool.tile([128, SLAB], bf16, tag=f"xtl{k}")
                    nc.sync.dma_start(
                        out=t1[:, :sl], in_=xt[k * 128 : (k + 1) * 128, s_lo : s_lo + sl]
                    )
                    xtl.append(t1)
                    if sh > 0:
                        t2 = xt_pool.tile([128, SLAB], bf16, tag=f"xth{k}")
                        nc.sync.dma_start(
                            out=t2[:, :sh],
                            in_=xt[k * 128 : (k + 1) * 128, s_hi : s_hi + sh],
                        )
                        xth.append(t2)
            if i % GRP == 0:
                g0 = i
                stg = xw_sb.tile([128, GRP * 128], bf16, tag="stg")
                if SIM_MEMSET:
                    nc.gpsimd.memset(stg[:], 0)
            if i % 2 == 0:
                ps = psum1.tile([128, 256], f32, tag="ps1")
            off = (i % 2) * 128
            lo0 = i * 128 - s_lo
            for k in range(n_kc):
                nc.tensor.matmul(
                    out=ps[:, off : off + f_out],
                    lhsT=xtl[k][:, lo0 : lo0 + 128],
                    rhs=w_t[k][:],
                    start=(k == 0),
                    stop=(k == n_kc - 1),
                )
            m_hi = min(128, n_hi - i * 128)
            if m_hi > 0:
                hi0 = (HALF + i * 128) - s_hi
                for k in range(n_kc):
                    nc.tensor.matmul(
                        out=ps[:m_hi, off + f_out : off + 128],
                        lhsT=xth[k][:, hi0 : hi0 + m_hi],
                        rhs=w_t[k][:],
                        start=(k == 0),
                        stop=(k == n_kc - 1),
                    )
            if i % 2 == 1 or i == NTL - 1:
                loc = ((i - 1 if i % 2 == 1 else i) - g0) * 128
                ncols = 256 if i % 2 == 1 else 128
                dst = stg[:, loc : loc + ncols]
                if (i // 2) % 2 == 0:
                    nc.scalar.activation(
                        out=dst, in_=ps[:, :ncols],
                        func=mybir.ActivationFunctionType.Copy,
                    )
                else:
                    nc.vector.tensor_copy(out=dst, in_=ps[:, :ncols])
            if i == NTL - 1 or (i + 1) % GRP == 0:
                gn = i + 1 - g0
                nc.sync.dma_start(
                    out=xw_pm[:, g0 * 128 : (g0 + gn) * 128], in_=stg[:, : gn * 128]
                )

        # ---- phase 2: multi-queue dma_gather + one-hot matmul segment-sum ----
        chunks = {}
        batches = {}

        def ensure_chunk(tile_idx):
            ci = tile_idx // CH
            if ci in chunks:
                return chunks[ci]
            q = ci % NQ
            g = gath[q].tile([128, CH, 128], bf16, tag=f"g{q}")
            nc.gpsimd.dma_gather(
                out_ap=g[:, :, :],
                in_ap=xw[:, :],
                idxs_ap=cols_t[:, ci * CH * 8 : (ci + 1) * CH * 8],
                num_idxs=CH * 128,
                num_idxs_reg=nreg,
                elem_size=128,
                single_packet=False,
                queue_num=q,
            )
            chunks[ci] = g
            return g

        def ensure_batch(tile_idx):
            bi = tile_idx // GB
            if bi in batches:
                return batches[bi]
            b0 = bi * GB
            g = ensure_chunk(b0)
            gs = b0 - (b0 // CH) * CH
            S_b = s_pool.tile([128, WIN, GB], bf16, tag="S")
            rhs_b = rhs_pool.tile([128, GB, f_out], bf16, tag="rhs")
            tmp_b = tmp_pool.tile([128, GB, f_out], bf16, tag="tmp")
            nc.vector.tensor_tensor(
                out=S_b[:],
                in0=meta_t[:, 3 * J : 3 * J + WIN * GB].rearrange(
                    "p (r b) -> p r b", b=GB
                ),
                in1=meta_t[:, b0 : b0 + GB]
                .rearrange("p (one b) -> p one b", one=1)
                .to_broadcast([128, WIN, GB]),
                op=mybir.AluOpType.is_equal,
            )
            nc.vector.tensor_tensor(
                out=rhs_b[:],
                in0=g[:, gs : gs + GB, 0:f_out],
                in1=meta_t[:, J + b0 : J + b0 + GB].to_broadcast([128, GB, f_out]),
                op=mybir.AluOpType.mult,
            )
            nc.vector.tensor_tensor(
                out=tmp_b[:],
                in0=g[:, gs : gs + GB, f_out:128],
                in1=meta_t[:, 2 * J + b0 : 2 * J + b0 + GB].to_broadcast(
                    [128, GB, f_out]
                ),
                op=mybir.AluOpType.mult,
            )
            batches[bi] = (S_b, rhs_b, tmp_b)
            return batches[bi]

        for w in range(n_win):
            cur_ps = psum2.tile([128, f_out], f32, tag="ps2")
            n_t = int(T[w])
            for k in range(n_t):
                t_s = int(starts[w]) + k
                S_b, rhs_b, tmp_b = ensure_batch(t_s)
                sl = t_s % GB
                lhsT = S_b[:, :, sl : sl + 1].rearrange("p r one -> p (r one)")
                nc.tensor.matmul(
                    out=cur_ps[:],
                    lhsT=lhsT,
                    rhs=rhs_b[:, sl : sl + 1, :],
                    start=(k == 0),
                    stop=False,
                )
                nc.tensor.matmul(
                    out=cur_ps[:],
                    lhsT=lhsT,
                    rhs=tmp_b[:, sl : sl + 1, :],
                    start=False,
                    stop=(k == n_t - 1),
                )
            rows = min(WIN, shard - w * WIN)
            ot = out_sb.tile([128, f_out], f32, tag="ot")
            nc.vector.tensor_copy(out=ot[:rows, :], in_=cur_ps[:rows, :])
            nc.sync.dma_start(out=out[w * WIN : w * WIN + rows, :], in_=ot[:rows, :])
    nc.compile()
    return nc


def kernel(X, W, edge_row, edge_col, edge_vals):
    global LAST_RESULTS
    X = np.asarray(X, dtype=np.float32)
    W = np.asarray(W, dtype=np.float32)
    edge_row = np.asarray(edge_row, dtype=np.int32)
    edge_col = np.asarray(edge_col, dtype=np.int32)
    edge_vals = np.asarray(edge_vals, dtype=np.float32)

    in_maps, T = _prep(X, W, edge_row, edge_col, edge_vals)
    nc = _build_nc(T)
    trace = TRACE and _install_ntff_hook()
    res = run_bass_kernel_spmd(
        nc, in_maps, core_ids=list(range(N_CORES)), trace=trace
    )
    LAST_RESULTS = res
    out = np.concatenate([res.results[p]["out"] for p in range(N_CORES)], axis=0)
    return out.astype(np.float32)
